# revision 10
# baseline (speedup 1.0000x reference)
"""JPEG layer (nn_JpegLayer) Trainium2 Bass kernel, 8-core data parallel.

Pipeline per image (per core: 4 images of [3,512,512]):
  P1: 3-accum f32r matmuls fold RGB->YCC color mix + H-DCT (+ vertical
      2x-pool for chroma) into [128,1024] 2-bank PSUM pairs. The Y drain
      (ACT Identity) subtracts sqrt(8)*L on h-freq DC rows = the -L level
      shift folded through the H-DCT.
  T1: PE transposes (f32r, identity rhs) -> [w, h-freq] pairs.
  P2: W-DCT (f32r). Chroma's 4 M=64 outputs pack into one [128,1024]
      pair via zero-padded [128,128] weight halves accumulated into the
      same region (the ISA rejects nonzero PSUM dst partition offsets).
  Q : all on DVE over [128,1024] pairs: e = d*(1/q) (TT, psum read,
      [128,512] table broadcast via stride-0 AP); r = (e + 1.5*2^23) -
      1.5*2^23 (dual-op tensor_scalar, bf16 out -- |r| < 256 so bf16 is
      exact); dec = r*q (bf16 TT).
  S3: fused W-IDCT + transpose as regular bf16 matmuls with dec chunks
      as the stationary operand (replaces P3 matmuls + T2 transposes).
  P4: bf16 N=1024 matmuls: H-IDCT + YCC->RGB fold (+ v-upsample chroma),
      one Y + one C matmul accumulated per [128,1024] psum tile.
  out: clamp [0,1] via DVE dual-op tensor_scalar, bf16 store on the
      scalar DGE ring (loads ride the sync ring), host upcasts to f32.

v2 scheduling fixes (baseline 121.1us):
  - consts packed into 3 dram tensors (f32r/bf16/f32) -> 3 DMAs, not 19.
  - input loads batched per (img,t): one [128, 3*512] DMA spanning all
    3 channels; img 0 spreads t across sync/scalar/gpsimd rings so both
    P1 j=0 tiles land in parallel.
  - PE warm-up: 8 throwaway N=512 matmuls on the const tile right after
    its DMA lands. The PE HAM clock gate defaults to K=4/8 (1.2 GHz) and
    only releases after ~3.4us of sustained activity; the baseline ran
    the whole first image's P1 at half clock (HAM warm only at 29.4us).
    Warming during the DMA head makes real work start at 2.4 GHz.
  - stores go on the gpsimd ring (sync keeps loads, scalar only consts +
    ACT drains), last image's store halves alternate gpsimd/sync.
"""
import os
import sys
sys.path.insert(0, '/opt/trn_rl_repo')
import numpy as np
import ml_dtypes
import concourse.bacc as bacc
import concourse.bass as bass
import concourse.mybir as mybir
import concourse.tile as tile
from concourse import bass_utils

N_CORES = 8
IMG_PER_CORE = 4
H = W = 512
LEVEL = np.float32(128.0 / 255.0)
C_ROUND = 12582912.0   # 1.5*2^23: (x+C)-C == round-half-even(x)
F32 = mybir.dt.float32
F32R = mybir.dt.float32r
BF16 = mybir.dt.bfloat16

RGB2YCC = np.array([[0.299, 0.587, 0.114],
                    [-0.168735892, -0.331264108, 0.5],
                    [0.5, -0.418687589, -0.081312411]], dtype=np.float32)
CB_C = np.array([0.0, -0.344136286, 1.772], dtype=np.float32)
CR_C = np.array([1.402, -0.714136286, 0.0], dtype=np.float32)

# offsets (in 128-col units) into the packed f32r const tiles: cfa holds
# the P1 weights (start-critical), cfb the T1/P2 weights
OFA = {"w1y0": 0, "w1y1": 1, "w1y2": 2, "w1c0": 3, "w1c1": 4, "w1c2": 5}
OFB = {"ident": 0, "w2y": 1, "w2c_lo": 2, "w2c_hi": 3}
NFA = 6 * 128
NFB = 4 * 128
# offsets into the packed bf16 const tile
OB = {"qt2b": (0, 512), "bdw_b": (512, 128), "pud2": (640, 256),
      "w4y_b": (896, 128), "w4cR_b": (1024, 128), "w4cG_b": (1152, 128),
      "w4cB_b": (1280, 128)}
NBF = 1408
# f32 tile: rqt2 [0:512], lneg [512:513], lpos [513:514]
NF32 = 514


def _dct8():
    i = np.arange(8)[:, None].astype(np.float64)
    j = np.arange(8)[None, :].astype(np.float64)
    m = np.sqrt(2.0 / 8) * np.cos(np.pi * (2 * j + 1) * i / 16.0)
    m[0, :] = 1.0 / np.sqrt(8.0)
    return m.astype(np.float32)


def _blockdiag(b, reps):
    r, c = b.shape
    out = np.zeros((r * reps, c * reps), dtype=np.float32)
    for k in range(reps):
        out[k * r:(k + 1) * r, k * c:(k + 1) * c] = b
    return out


def _build_consts(quantize):
    D = _dct8()
    BD_T = _blockdiag(D.T, 16)             # [128,128] fwd 1D-DCT as lhsT
    BD = _blockdiag(D, 16)                 # [128,128] inverse
    pf8 = np.zeros((16, 8), dtype=np.float32)
    for ii in range(8):
        for dh in range(2):
            pf8[2 * ii + dh, :] = D[:, ii] * 0.5
    PF = _blockdiag(pf8, 8)                # [128, 64]
    pu8 = np.zeros((8, 16), dtype=np.float32)
    for jj in range(8):
        for dw in range(2):
            pu8[:, 2 * jj + dw] = D[:, jj]
    PU = _blockdiag(pu8, 8)                # [64, 128]

    bf = ml_dtypes.bfloat16

    fa = np.zeros((128, NFA), dtype=np.float32)
    fb = np.zeros((128, NFB), dtype=np.float32)
    def putf(name, arr):
        if name in OFA:
            fa[:, OFA[name] * 128:OFA[name] * 128 + arr.shape[1]] = arr
        else:
            fb[:, OFB[name] * 128:OFB[name] * 128 + arr.shape[1]] = arr
    for c in range(3):
        putf(f"w1y{c}", RGB2YCC[0, c] * BD_T)
        putf(f"w1c{c}", np.concatenate(
            [RGB2YCC[1, c] * PF, RGB2YCC[2, c] * PF], axis=1))
    putf("ident", np.eye(128, dtype=np.float32))
    putf("w2y", BD_T)
    w2c_lo = np.zeros((128, 128), dtype=np.float32)
    w2c_lo[:, 0:64] = PF
    w2c_hi = np.zeros((128, 128), dtype=np.float32)
    w2c_hi[:, 64:128] = PF
    putf("w2c_lo", w2c_lo)
    putf("w2c_hi", w2c_hi)

    q = (np.round(quantize[0].astype(np.float32) * np.float32(255.0))
         / np.float32(255.0)).astype(np.float32)
    rq = (1.0 / q.astype(np.float64)).astype(np.float32)

    bfc = np.zeros((128, NBF), dtype=np.float32)
    def putb(name, arr):
        o, n = OB[name]
        assert arr.shape[1] == n
        bfc[:, o:o + n] = arr
    putb("qt2b", np.tile(q.T, (16, 64)))
    putb("bdw_b", BD)
    pud2 = np.zeros((128, 256), dtype=np.float32)
    pud2[0:64, 0:128] = PU
    pud2[64:128, 128:256] = PU
    putb("pud2", pud2)
    putb("w4y_b", BD)
    for name, cb, cr in (("R", CB_C[0], CR_C[0]), ("G", CB_C[1], CR_C[1]),
                         ("B", CB_C[2], CR_C[2])):
        m = np.zeros((128, 128), dtype=np.float32)
        m[0:64, :] = cb * PU
        m[64:128, :] = cr * PU
        putb(f"w4c{name}_b", m)

    f32c = np.zeros((128, NF32), dtype=np.float32)
    f32c[:, 0:512] = np.tile(rq.T, (16, 64))
    f32c[0::8, 512] = -np.float32(np.sqrt(8.0) * LEVEL)
    f32c[0::8, 513] = np.float32(np.sqrt(8.0) * LEVEL)

    return {"cfa": fa, "cfb": fb, "cbf": bfc.astype(bf), "cf32": f32c}


def _build_nc():
    nc = bacc.Bacc("TRN2", target_bir_lowering=False, debug=False,
                   enable_asserts=False, num_devices=N_CORES)
    x_d = nc.dram_tensor("x", [IMG_PER_CORE, 3, H, W], F32R,
                         kind="ExternalInput").ap()
    out_d = nc.dram_tensor("out", [IMG_PER_CORE, 3, H, W], BF16,
                           kind="ExternalOutput").ap()
    cfa_d = nc.dram_tensor("cfa", [128, NFA], F32R,
                           kind="ExternalInput").ap()
    cfb_d = nc.dram_tensor("cfb", [128, NFB], F32R,
                           kind="ExternalInput").ap()
    cbf_d = nc.dram_tensor("cbf", [128, NBF], BF16,
                           kind="ExternalInput").ap()
    cf32_d = nc.dram_tensor("cf32", [128, NF32], F32,
                            kind="ExternalInput").ap()

    ACT = mybir.ActivationFunctionType
    OP = mybir.AluOpType

    with tile.TileContext(nc) as tc:
        with tc.tile_pool(name="consts", bufs=1) as cp, \
             tc.tile_pool(name="xin", bufs=10) as xp, \
             tc.tile_pool(name="fwd", bufs=4) as fp, \
             tc.tile_pool(name="qnt", bufs=4) as qp, \
             tc.tile_pool(name="dcd", bufs=3) as dp, \
             tc.tile_pool(name="outp", bufs=8) as op_, \
             tc.tile_pool(name="psmm", bufs=2, space="PSUM") as pmm, \
             tc.tile_pool(name="pstp", bufs=2, space="PSUM") as ptp:

            # ---- packed const loads, all on the scalar ring in order of
            # first use: P1 weights, T1/P2 weights, q tables, bf16 weights
            cfa = cp.tile([128, NFA], F32R, tag="cfa", name="cfa")
            nc.scalar.dma_start(cfa[:], cfa_d)
            cfb = cp.tile([128, NFB], F32R, tag="cfb", name="cfb")
            nc.scalar.dma_start(cfb[:], cfb_d)
            cf32 = cp.tile([128, NF32], F32, tag="cf32", name="cf32")
            nc.scalar.dma_start(cf32[:], cf32_d)
            cbf = cp.tile([128, NBF], BF16, tag="cbf", name="cbf")
            nc.scalar.dma_start(cbf[:], cbf_d)

            def CW(name):  # [128,128] f32r const slice
                if name in OFA:
                    return cfa[:, OFA[name] * 128:OFA[name] * 128 + 128]
                return cfb[:, OFB[name] * 128:OFB[name] * 128 + 128]

            def CB(name):  # bf16 const slice
                o, n = OB[name]
                return cbf[:, o:o + n]

            rqt2 = cf32[:, 0:512]
            lneg = cf32[:, 512:513]
            lpos = cf32[:, 513:514]

            # ---- PE warm-up: release the HAM clock gate during the DMA
            # head. The PE defaults to K=4/8 (1.2 GHz) and only unthrottles
            # after ~3.4us of sustained activity; throwaway N=512 matmuls
            # over a memset tile (no DMA dependency -- PE is busy right out
            # of the framework preamble) flip it before real work arrives.
            junk = cp.tile([128, 512], BF16, tag="junk", name="junk")
            nc.gpsimd.memset(junk[:], 0.0)
            warm = ptp.tile([128, 1024], F32, tag="tp", name="pstp")
            for k in range(9):
                nc.tensor.matmul(warm[:, 0:512], junk[:, 0:128],
                                 junk[:], start=True, stop=True)

            for img in range(IMG_PER_CORE):
                # ---- batched RGB loads: one DMA per (img, t) spanning all
                # 3 channels; img 0 fans out across 3 DGE rings ----
                X = {}
                for t in range(4):
                    if img == 0 and t < 2:
                        # per-channel pieces so P1 j=0 can start on the
                        # first [128,512] to land; t0 on sync, t1 gpsimd
                        eng = nc.sync if t == 0 else nc.gpsimd
                        xt = xp.tile([128, 1536], F32R, tag="x",
                                     name=f"x_{img}_{t}")
                        for c in range(3):
                            eng.dma_start(xt[:, 512 * c:512 * (c + 1)],
                                          x_d[img, c,
                                              128 * t:128 * (t + 1), :])
                            X[c, t] = xt[:, 512 * c:512 * (c + 1)]
                        continue
                    xt = xp.tile([128, 1536], F32R, tag="x",
                                 name=f"x_{img}_{t}")
                    if img == 0:
                        eng = nc.sync if t == 2 else nc.gpsimd
                    else:
                        eng = nc.sync
                    src = x_d[img].rearrange("c (t p) w -> t p c w",
                                             p=128)[t]
                    eng.dma_start(
                        xt[:].rearrange("p (c w) -> p c w", c=3), src)
                    for c in range(3):
                        X[c, t] = xt[:, 512 * c:512 * (c + 1)]

                # ---- P1: color + H-DCT (+v-pool chroma), pairs over t ----
                d1y, d1c = [], []
                for j in range(2):
                    psY = ptp.tile([128, 1024], F32, tag="tp", name="pstp")
                    for b in range(2):
                        t = 2 * j + b
                        for c in range(3):
                            nc.tensor.matmul(psY[:, 512 * b:512 * (b + 1)],
                                             CW(f"w1y{c}"), X[c, t],
                                             start=(c == 0), stop=(c == 2))
                    ty = fp.tile([128, 1024], F32R, tag="d1y",
                                 name=f"d1y_{img}_{j}")
                    nc.scalar.activation(ty[:], psY[:], ACT.Identity,
                                         bias=lneg)
                    d1y.append(ty)
                    psC = ptp.tile([128, 1024], F32, tag="tp", name="pstp")
                    for b in range(2):
                        t = 2 * j + b
                        for c in range(3):
                            nc.tensor.matmul(psC[:, 512 * b:512 * (b + 1)],
                                             CW(f"w1c{c}"), X[c, t],
                                             start=(c == 0), stop=(c == 2))
                    tcc = fp.tile([128, 1024], F32R, tag="d1c",
                                  name=f"d1c_{img}_{j}")
                    nc.scalar.activation(tcc[:], psC[:], ACT.Copy)
                    d1c.append(tcc)

                # ---- T1: PE transposes, pairs over s ----
                t1y, t1c = [], []
                for u in range(2):
                    pty = ptp.tile([128, 1024], F32R, tag="tp", name="pstp")
                    for b in range(2):
                        s = 2 * u + b
                        for t in range(4):
                            nc.tensor.transpose(
                                pty[:, 512 * b + 128 * t:512 * b + 128 * (t + 1)],
                                d1y[t // 2][:, 512 * (t % 2) + 128 * s:
                                            512 * (t % 2) + 128 * (s + 1)],
                                CW("ident"))
                    sy = fp.tile([128, 1024], F32R, tag="t1y",
                                 name=f"t1y_{img}_{u}")
                    nc.scalar.activation(sy[:], pty[:], ACT.Copy)
                    t1y.append(sy)
                for u in range(2):
                    ptc = ptp.tile([128, 1024], F32R, tag="tp", name="pstp")
                    for b in range(2):
                        s = 2 * u + b
                        for t in range(4):
                            nc.tensor.transpose(
                                ptc[:, 512 * b + 128 * t:512 * b + 128 * (t + 1)],
                                d1c[t // 2][:, 512 * (t % 2) + 128 * s:
                                            512 * (t % 2) + 128 * (s + 1)],
                                CW("ident"))
                    sc = fp.tile([128, 1024], F32R, tag="t1c",
                                 name=f"t1c_{img}_{u}")
                    nc.scalar.activation(sc[:], ptc[:], ACT.Copy)
                    t1c.append(sc)

                # ---- P2 + quantize (all DVE) ----
                decy = []
                for u in range(2):
                    ps = ptp.tile([128, 1024], F32, tag="tp", name="pstp")
                    for b in range(2):
                        nc.tensor.matmul(ps[:, 512 * b:512 * (b + 1)],
                                         CW("w2y"),
                                         t1y[u][:, 512 * b:512 * (b + 1)],
                                         start=True, stop=True)
                    ey = qp.tile([128, 1024], F32, tag="ey",
                                 name=f"ey_{img}_{u}")
                    ry = qp.tile([128, 1024], BF16, tag="ry",
                                 name=f"ry_{img}_{u}")
                    dy = dp.tile([128, 1024], BF16, tag="decy",
                                 name=f"decy_{img}_{u}")
                    if img == IMG_PER_CORE - 1:
                        for h in range(2):
                            sl = slice(512 * h, 512 * (h + 1))
                            ve = nc.vector if h == 0 else nc.gpsimd
                            nc.vector.tensor_tensor(ey[:, sl], ps[:, sl],
                                                    rqt2, OP.mult)
                            ve.tensor_scalar(ry[:, sl], ey[:, sl],
                                             C_ROUND, C_ROUND,
                                             OP.add, OP.subtract)
                            ve.tensor_tensor(dy[:, sl], ry[:, sl],
                                             CB("qt2b"), OP.mult)
                    else:
                        nc.vector.tensor_tensor(
                            ey[:].rearrange("p (b w) -> p b w", b=2),
                            ps[:].rearrange("p (b w) -> p b w", b=2),
                            rqt2.unsqueeze(1)
                            .broadcast_to([128, 2, 512]), OP.mult)
                        nc.vector.tensor_scalar(ry[:], ey[:], C_ROUND,
                                                C_ROUND, OP.add, OP.subtract)
                        nc.vector.tensor_tensor(
                            dy[:].rearrange("p (b w) -> p b w", b=2),
                            ry[:].rearrange("p (b w) -> p b w", b=2),
                            CB("qt2b").unsqueeze(1)
                            .broadcast_to([128, 2, 512]), OP.mult)
                    decy.append(dy)

                psc = ptp.tile([128, 1024], F32, tag="tp", name="pstp")
                for s in range(4):
                    nc.tensor.matmul(
                        psc[:, 512 * (s // 2):512 * (s // 2) + 512],
                        CW("w2c_hi" if s % 2 else "w2c_lo"),
                        t1c[s // 2][:, 512 * (s % 2):512 * (s % 2) + 512],
                        start=(s % 2 == 0), stop=(s % 2 == 1))
                ec = qp.tile([128, 1024], F32, tag="ey", name=f"ec_{img}")
                rc = qp.tile([128, 1024], BF16, tag="ry", name=f"rc_{img}")
                decc = dp.tile([128, 1024], BF16, tag="decc",
                               name=f"decc_{img}")
                if img == IMG_PER_CORE - 1:
                    for h in range(2):
                        sl = slice(512 * h, 512 * (h + 1))
                        ve = nc.vector if h == 0 else nc.gpsimd
                        nc.vector.tensor_tensor(ec[:, sl], psc[:, sl],
                                                rqt2, OP.mult)
                        ve.tensor_scalar(rc[:, sl], ec[:, sl],
                                         C_ROUND, C_ROUND,
                                         OP.add, OP.subtract)
                        ve.tensor_tensor(decc[:, sl], rc[:, sl],
                                         CB("qt2b"), OP.mult)
                else:
                    nc.vector.tensor_tensor(
                        ec[:].rearrange("p (b w) -> p b w", b=2),
                        psc[:].rearrange("p (b w) -> p b w", b=2),
                        rqt2.unsqueeze(1)
                        .broadcast_to([128, 2, 512]), OP.mult)
                    nc.vector.tensor_scalar(rc[:], ec[:], C_ROUND, C_ROUND,
                                            OP.add, OP.subtract)
                    nc.vector.tensor_tensor(
                        decc[:].rearrange("p (b w) -> p b w", b=2),
                        rc[:].rearrange("p (b w) -> p b w", b=2),
                        CB("qt2b").unsqueeze(1)
                        .broadcast_to([128, 2, 512]), OP.mult)

                # ---- S3: fused W-IDCT + transpose (bf16 matmuls) ----
                t2y, t2c = [], []
                for v in range(2):
                    ps = pmm.tile([128, 1024], F32, tag="mm", name="psmm")
                    for b in range(2):
                        t = 2 * v + b
                        for s in range(4):
                            nc.tensor.matmul(
                                ps[:, 512 * b + 128 * s:512 * b + 128 * (s + 1)],
                                decy[s // 2][:, 512 * (s % 2) + 128 * t:
                                             512 * (s % 2) + 128 * (t + 1)],
                                CB("bdw_b"), start=True, stop=True)
                    sy = dp.tile([128, 1024], BF16, tag="t2y",
                                 name=f"t2y_{img}_{v}")
                    nc.scalar.activation(sy[:], ps[:], ACT.Identity,
                                         bias=lpos)
                    t2y.append(sy)
                for v in range(2):
                    ps = pmm.tile([128, 1024], F32, tag="mm", name="psmm")
                    for b in range(2):
                        t = 2 * v + b
                        for g in range(2):
                            nc.tensor.matmul(
                                ps[:, 512 * b + 256 * g:512 * b + 256 * (g + 1)],
                                decc[:, 512 * g + 128 * t:
                                     512 * g + 128 * (t + 1)],
                                CB("pud2"), start=True, stop=True)
                    sc = dp.tile([128, 1024], BF16, tag="t2c",
                                 name=f"t2c_{img}_{v}")
                    nc.scalar.activation(sc[:], ps[:], ACT.Copy)
                    t2c.append(sc)

                # ---- P4: H-IDCT + color + clamp + store ----
                for ci, cname in enumerate(("R", "G", "B")):
                    for v in range(2):
                        if img == IMG_PER_CORE - 1 and (2 * ci + v) % 2:
                            ps = ptp.tile([128, 1024], F32, tag="tp",
                                          name="pstp")
                        else:
                            ps = pmm.tile([128, 1024], F32, tag="mm",
                                          name="psmm")
                        for b in range(2):
                            nc.tensor.matmul(
                                ps[:, 512 * b:512 * (b + 1)], CB("w4y_b"),
                                t2y[v][:, 512 * b:512 * (b + 1)],
                                start=True, stop=False)
                            nc.tensor.matmul(
                                ps[:, 512 * b:512 * (b + 1)],
                                CB(f"w4c{cname}_b"),
                                t2c[v][:, 512 * b:512 * (b + 1)],
                                start=False, stop=True)
                        og = op_.tile([128, 1024], BF16, tag="og",
                                      name=f"og_{img}_{ci}_{v}")
                        if img == IMG_PER_CORE - 1:
                            # last image: clamp+store per half; halves
                            # alternate scalar/sync rings so the tail
                            # drains on two queues
                            for b in range(2):
                                sl = slice(512 * b, 512 * (b + 1))
                                nc.vector.tensor_scalar(og[:, sl], ps[:, sl],
                                                        0.0, 1.0,
                                                        OP.max, OP.min)
                                t = 2 * v + b
                                eng = nc.gpsimd if b == 0 else nc.sync
                                eng.dma_start(
                                    out_d[img, ci, 128 * t:128 * (t + 1), :],
                                    og[:, sl])
                        else:
                            nc.vector.tensor_scalar(og[:], ps[:], 0.0, 1.0,
                                                    OP.max, OP.min)
                            nc.gpsimd.dma_start(
                                out_d[img, ci, 256 * v:256 * (v + 1), :]
                                .rearrange("(b p) w -> p b w", b=2),
                                og[:].rearrange("p (b w) -> p b w", b=2))
    nc.compile()
    return nc


_NC_CACHE = None


def kernel(input, quantize):
    global _NC_CACHE
    input = np.asarray(input, dtype=np.float32)
    quantize = np.asarray(quantize, dtype=np.float32)
    consts = _build_consts(quantize)
    if _NC_CACHE is None:
        _NC_CACHE = _build_nc()
    nc = _NC_CACHE

    in_maps = []
    for core in range(N_CORES):
        shard = np.ascontiguousarray(
            input[core * IMG_PER_CORE:(core + 1) * IMG_PER_CORE])
        m = {"x": shard}
        m.update(consts)
        in_maps.append(m)
    trace = bool(os.environ.get("JPEG_TRACE"))
    kw = {}
    if trace:
        kw["trace"] = True
        td = os.environ.get("JPEG_TRACE_DIR")
        if td:
            os.makedirs(td, exist_ok=True)
            kw["tmpdir"] = td
    res = bass_utils.run_bass_kernel_spmd(nc, in_maps,
                                          core_ids=list(range(N_CORES)), **kw)
    global LAST_RESULT
    LAST_RESULT = res
    out = np.concatenate(
        [np.asarray(res.results[i]["out"]) for i in range(N_CORES)], axis=0)
    return out.astype(np.float32)


LAST_RESULT = None


# revision 11
# speedup vs baseline: 1.1453x; 1.1453x over previous
"""JPEG layer (nn_JpegLayer) Trainium2 Bass kernel, 8-core data parallel.

Pipeline per image (per core: 4 images of [3,512,512]):
  P1: 3-accum f32r matmuls fold RGB->YCC color mix + H-DCT (+ vertical
      2x-pool for chroma) into [128,1024] 2-bank PSUM pairs. The Y drain
      (ACT Identity) subtracts sqrt(8)*L on h-freq DC rows = the -L level
      shift folded through the H-DCT.
  T1: PE transposes (f32r, identity rhs) -> [w, h-freq] pairs.
  P2: W-DCT (f32r). Chroma's 4 M=64 outputs pack into one [128,1024]
      pair via zero-padded [128,128] weight halves accumulated into the
      same region (the ISA rejects nonzero PSUM dst partition offsets).
  Q : all on DVE over [128,1024] pairs: e = d*(1/q) (TT, psum read,
      [128,512] table broadcast via stride-0 AP); r = (e + 1.5*2^23) -
      1.5*2^23 (dual-op tensor_scalar, bf16 out -- |r| < 256 so bf16 is
      exact); dec = r*q (bf16 TT).
  S3: fused W-IDCT + transpose as regular bf16 matmuls with dec chunks
      as the stationary operand (replaces P3 matmuls + T2 transposes).
  P4: bf16 N=1024 matmuls: H-IDCT + YCC->RGB fold (+ v-upsample chroma),
      one Y + one C matmul accumulated per [128,1024] psum tile.
  out: clamp [0,1] via DVE dual-op tensor_scalar, bf16 store on the
      scalar DGE ring (loads ride the sync ring), host upcasts to f32.

v2 scheduling fixes (baseline 121.1us):
  - consts packed into 3 dram tensors (f32r/bf16/f32) -> 3 DMAs, not 19.
  - input loads batched per (img,t): one [128, 3*512] DMA spanning all
    3 channels; img 0 spreads t across sync/scalar/gpsimd rings so both
    P1 j=0 tiles land in parallel.
  - PE warm-up: 8 throwaway N=512 matmuls on the const tile right after
    its DMA lands. The PE HAM clock gate defaults to K=4/8 (1.2 GHz) and
    only releases after ~3.4us of sustained activity; the baseline ran
    the whole first image's P1 at half clock (HAM warm only at 29.4us).
    Warming during the DMA head makes real work start at 2.4 GHz.
  - stores go on the gpsimd ring (sync keeps loads, scalar only consts +
    ACT drains), last image's store halves alternate gpsimd/sync.
"""
import os
import sys
sys.path.insert(0, '/opt/trn_rl_repo')
import numpy as np
import ml_dtypes
import concourse.bacc as bacc
import concourse.bass as bass
import concourse.mybir as mybir
import concourse.tile as tile
from concourse import bass_utils

N_CORES = 8
IMG_PER_CORE = 4
H = W = 512
LEVEL = np.float32(128.0 / 255.0)
C_ROUND = 12582912.0   # 1.5*2^23: (x+C)-C == round-half-even(x)
F32 = mybir.dt.float32
F32R = mybir.dt.float32r
BF16 = mybir.dt.bfloat16

RGB2YCC = np.array([[0.299, 0.587, 0.114],
                    [-0.168735892, -0.331264108, 0.5],
                    [0.5, -0.418687589, -0.081312411]], dtype=np.float32)
CB_C = np.array([0.0, -0.344136286, 1.772], dtype=np.float32)
CR_C = np.array([1.402, -0.714136286, 0.0], dtype=np.float32)

# offsets (in 128-col units) into the packed f32r const tiles: cfa holds
# the P1 weights (start-critical), cfb the T1/P2 weights
OFA = {"w1y0": 0, "w1y1": 1, "w1y2": 2, "w1c0": 3, "w1c1": 4, "w1c2": 5}
OFB = {"ident": 0, "w2y": 1, "w2c_lo": 2, "w2c_hi": 3}
NFA = 6 * 128
NFB = 4 * 128
# offsets into the packed bf16 const tile
OB = {"qt2b": (0, 512), "bdw_b": (512, 128), "pud2": (640, 256),
      "w4y_b": (896, 128), "w4cR_b": (1024, 128), "w4cG_b": (1152, 128),
      "w4cB_b": (1280, 128)}
NBF = 1408
# f32 tile: rqt2 [0:512], lneg [512:513], lpos [513:514]
NF32 = 514


def _dct8():
    i = np.arange(8)[:, None].astype(np.float64)
    j = np.arange(8)[None, :].astype(np.float64)
    m = np.sqrt(2.0 / 8) * np.cos(np.pi * (2 * j + 1) * i / 16.0)
    m[0, :] = 1.0 / np.sqrt(8.0)
    return m.astype(np.float32)


def _blockdiag(b, reps):
    r, c = b.shape
    out = np.zeros((r * reps, c * reps), dtype=np.float32)
    for k in range(reps):
        out[k * r:(k + 1) * r, k * c:(k + 1) * c] = b
    return out


def _build_consts(quantize):
    D = _dct8()
    BD_T = _blockdiag(D.T, 16)             # [128,128] fwd 1D-DCT as lhsT
    BD = _blockdiag(D, 16)                 # [128,128] inverse
    pf8 = np.zeros((16, 8), dtype=np.float32)
    for ii in range(8):
        for dh in range(2):
            pf8[2 * ii + dh, :] = D[:, ii] * 0.5
    PF = _blockdiag(pf8, 8)                # [128, 64]
    pu8 = np.zeros((8, 16), dtype=np.float32)
    for jj in range(8):
        for dw in range(2):
            pu8[:, 2 * jj + dw] = D[:, jj]
    PU = _blockdiag(pu8, 8)                # [64, 128]

    bf = ml_dtypes.bfloat16

    fa = np.zeros((128, NFA), dtype=np.float32)
    fb = np.zeros((128, NFB), dtype=np.float32)
    def putf(name, arr):
        if name in OFA:
            fa[:, OFA[name] * 128:OFA[name] * 128 + arr.shape[1]] = arr
        else:
            fb[:, OFB[name] * 128:OFB[name] * 128 + arr.shape[1]] = arr
    for c in range(3):
        putf(f"w1y{c}", RGB2YCC[0, c] * BD_T)
        putf(f"w1c{c}", np.concatenate(
            [RGB2YCC[1, c] * PF, RGB2YCC[2, c] * PF], axis=1))
    putf("ident", np.eye(128, dtype=np.float32))
    putf("w2y", BD_T)
    w2c_lo = np.zeros((128, 128), dtype=np.float32)
    w2c_lo[:, 0:64] = PF
    w2c_hi = np.zeros((128, 128), dtype=np.float32)
    w2c_hi[:, 64:128] = PF
    putf("w2c_lo", w2c_lo)
    putf("w2c_hi", w2c_hi)

    q = (np.round(quantize[0].astype(np.float32) * np.float32(255.0))
         / np.float32(255.0)).astype(np.float32)
    rq = (1.0 / q.astype(np.float64)).astype(np.float32)

    bfc = np.zeros((128, NBF), dtype=np.float32)
    def putb(name, arr):
        o, n = OB[name]
        assert arr.shape[1] == n
        bfc[:, o:o + n] = arr
    putb("qt2b", np.tile(q.T, (16, 64)))
    putb("bdw_b", BD)
    pud2 = np.zeros((128, 256), dtype=np.float32)
    pud2[0:64, 0:128] = PU
    pud2[64:128, 128:256] = PU
    putb("pud2", pud2)
    putb("w4y_b", BD)
    for name, cb, cr in (("R", CB_C[0], CR_C[0]), ("G", CB_C[1], CR_C[1]),
                         ("B", CB_C[2], CR_C[2])):
        m = np.zeros((128, 128), dtype=np.float32)
        m[0:64, :] = cb * PU
        m[64:128, :] = cr * PU
        putb(f"w4c{name}_b", m)

    f32c = np.zeros((128, NF32), dtype=np.float32)
    f32c[:, 0:512] = np.tile(rq.T, (16, 64))
    f32c[0::8, 512] = -np.float32(np.sqrt(8.0) * LEVEL)
    f32c[0::8, 513] = np.float32(np.sqrt(8.0) * LEVEL)

    return {"cfa": fa, "cfb": fb, "cbf": bfc.astype(bf), "cf32": f32c}


def _build_nc():
    nc = bacc.Bacc("TRN2", target_bir_lowering=False, debug=False,
                   enable_asserts=False, num_devices=N_CORES)
    x_d = nc.dram_tensor("x", [IMG_PER_CORE, 3, H, W], F32R,
                         kind="ExternalInput").ap()
    out_d = nc.dram_tensor("out", [IMG_PER_CORE, 3, H, W], BF16,
                           kind="ExternalOutput").ap()
    cfa_d = nc.dram_tensor("cfa", [128, NFA], F32R,
                           kind="ExternalInput").ap()
    cfb_d = nc.dram_tensor("cfb", [128, NFB], F32R,
                           kind="ExternalInput").ap()
    cbf_d = nc.dram_tensor("cbf", [128, NBF], BF16,
                           kind="ExternalInput").ap()
    cf32_d = nc.dram_tensor("cf32", [128, NF32], F32,
                            kind="ExternalInput").ap()

    ACT = mybir.ActivationFunctionType
    OP = mybir.AluOpType

    with tile.TileContext(nc) as tc:
        with tc.tile_pool(name="consts", bufs=1) as cp, \
             tc.tile_pool(name="xin", bufs=10) as xp, \
             tc.tile_pool(name="fwd", bufs=4) as fp, \
             tc.tile_pool(name="qnt", bufs=4) as qp, \
             tc.tile_pool(name="dcd", bufs=3) as dp, \
             tc.tile_pool(name="outp", bufs=8) as op_, \
             tc.tile_pool(name="psmm", bufs=2, space="PSUM") as pmm, \
             tc.tile_pool(name="pstp", bufs=2, space="PSUM") as ptp:

            # ---- packed const loads, all on the scalar ring in order of
            # first use: P1 weights, T1/P2 weights, q tables, bf16 weights
            cfa = cp.tile([128, NFA], F32R, tag="cfa", name="cfa")
            nc.scalar.dma_start(cfa[:], cfa_d)
            cfb = cp.tile([128, NFB], F32R, tag="cfb", name="cfb")
            nc.scalar.dma_start(cfb[:], cfb_d)
            cf32 = cp.tile([128, NF32], F32, tag="cf32", name="cf32")
            nc.scalar.dma_start(cf32[:], cf32_d)
            cbf = cp.tile([128, NBF], BF16, tag="cbf", name="cbf")
            nc.scalar.dma_start(cbf[:], cbf_d)

            def CW(name):  # [128,128] f32r const slice
                if name in OFA:
                    return cfa[:, OFA[name] * 128:OFA[name] * 128 + 128]
                return cfb[:, OFB[name] * 128:OFB[name] * 128 + 128]

            def CB(name):  # bf16 const slice
                o, n = OB[name]
                return cbf[:, o:o + n]

            rqt2 = cf32[:, 0:512]
            lneg = cf32[:, 512:513]
            lpos = cf32[:, 513:514]

            # ---- PE warm-up: release the HAM clock gate during the DMA
            # head. The PE defaults to K=4/8 (1.2 GHz) and only unthrottles
            # after ~3.4us of sustained activity; throwaway N=512 matmuls
            # over a memset tile (no DMA dependency -- PE is busy right out
            # of the framework preamble) flip it before real work arrives.
            junk = cp.tile([128, 512], BF16, tag="junk", name="junk")
            nc.gpsimd.memset(junk[:], 0.0)
            warm = ptp.tile([128, 1024], F32, tag="tp", name="pstp")
            for k in range(9):
                nc.tensor.matmul(warm[:, 0:512], junk[:, 0:128],
                                 junk[:], start=True, stop=True)

            for img in range(IMG_PER_CORE):
                # ---- batched RGB loads: one DMA per (img, t) spanning all
                # 3 channels; img 0 fans out across 3 DGE rings ----
                X = {}
                for t in range(4):
                    if img == 0 and t < 2:
                        # per-channel pieces so P1 j=0 can start on the
                        # first [128,512] to land; t0 on sync, t1 gpsimd
                        eng = nc.sync if t == 0 else nc.gpsimd
                        xt = xp.tile([128, 1536], F32R, tag="x",
                                     name=f"x_{img}_{t}")
                        for c in range(3):
                            eng.dma_start(xt[:, 512 * c:512 * (c + 1)],
                                          x_d[img, c,
                                              128 * t:128 * (t + 1), :])
                            X[c, t] = xt[:, 512 * c:512 * (c + 1)]
                        continue
                    xt = xp.tile([128, 1536], F32R, tag="x",
                                 name=f"x_{img}_{t}")
                    if img == 0:
                        eng = nc.sync if t == 2 else nc.gpsimd
                    else:
                        eng = nc.sync
                    src = x_d[img].rearrange("c (t p) w -> t p c w",
                                             p=128)[t]
                    eng.dma_start(
                        xt[:].rearrange("p (c w) -> p c w", c=3), src)
                    for c in range(3):
                        X[c, t] = xt[:, 512 * c:512 * (c + 1)]

                # ---- P1: color + H-DCT (+v-pool chroma), pairs over t ----
                d1y, d1c = [], []
                for j in range(2):
                    psY = ptp.tile([128, 1024], F32, tag="tp", name="pstp")
                    for b in range(2):
                        t = 2 * j + b
                        for c in range(3):
                            nc.tensor.matmul(psY[:, 512 * b:512 * (b + 1)],
                                             CW(f"w1y{c}"), X[c, t],
                                             start=(c == 0), stop=(c == 2))
                    ty = fp.tile([128, 1024], F32R, tag="d1y",
                                 name=f"d1y_{img}_{j}")
                    nc.scalar.activation(ty[:], psY[:], ACT.Identity,
                                         bias=lneg)
                    d1y.append(ty)
                    psC = ptp.tile([128, 1024], F32, tag="tp", name="pstp")
                    for b in range(2):
                        t = 2 * j + b
                        for c in range(3):
                            nc.tensor.matmul(psC[:, 512 * b:512 * (b + 1)],
                                             CW(f"w1c{c}"), X[c, t],
                                             start=(c == 0), stop=(c == 2))
                    tcc = fp.tile([128, 1024], F32R, tag="d1c",
                                  name=f"d1c_{img}_{j}")
                    nc.scalar.activation(tcc[:], psC[:], ACT.Copy)
                    d1c.append(tcc)

                # ---- T1: PE transposes, pairs over s ----
                t1y, t1c = [], []
                for u in range(2):
                    pty = ptp.tile([128, 1024], F32R, tag="tp", name="pstp")
                    for b in range(2):
                        s = 2 * u + b
                        for t in range(4):
                            nc.tensor.transpose(
                                pty[:, 512 * b + 128 * t:512 * b + 128 * (t + 1)],
                                d1y[t // 2][:, 512 * (t % 2) + 128 * s:
                                            512 * (t % 2) + 128 * (s + 1)],
                                CW("ident"))
                    sy = fp.tile([128, 1024], F32R, tag="t1y",
                                 name=f"t1y_{img}_{u}")
                    nc.scalar.activation(sy[:], pty[:], ACT.Copy)
                    t1y.append(sy)
                for u in range(2):
                    ptc = ptp.tile([128, 1024], F32R, tag="tp", name="pstp")
                    for b in range(2):
                        s = 2 * u + b
                        for t in range(4):
                            nc.tensor.transpose(
                                ptc[:, 512 * b + 128 * t:512 * b + 128 * (t + 1)],
                                d1c[t // 2][:, 512 * (t % 2) + 128 * s:
                                            512 * (t % 2) + 128 * (s + 1)],
                                CW("ident"))
                    sc = fp.tile([128, 1024], F32R, tag="t1c",
                                 name=f"t1c_{img}_{u}")
                    nc.scalar.activation(sc[:], ptc[:], ACT.Copy)
                    t1c.append(sc)

                # ---- P2 + quantize (all DVE) ----
                decy = []
                for u in range(2):
                    ps = ptp.tile([128, 1024], F32, tag="tp", name="pstp")
                    for b in range(2):
                        nc.tensor.matmul(ps[:, 512 * b:512 * (b + 1)],
                                         CW("w2y"),
                                         t1y[u][:, 512 * b:512 * (b + 1)],
                                         start=True, stop=True)
                    ey = qp.tile([128, 1024], F32, tag="ey",
                                 name=f"ey_{img}_{u}")
                    ry = qp.tile([128, 1024], BF16, tag="ry",
                                 name=f"ry_{img}_{u}")
                    dy = dp.tile([128, 1024], BF16, tag="decy",
                                 name=f"decy_{img}_{u}")
                    if img == IMG_PER_CORE - 1:
                        for h in range(2):
                            sl = slice(512 * h, 512 * (h + 1))
                            nc.vector.tensor_tensor(ey[:, sl], ps[:, sl],
                                                    rqt2, OP.mult)
                            nc.vector.tensor_scalar(ry[:, sl], ey[:, sl],
                                                    C_ROUND, C_ROUND,
                                                    OP.add, OP.subtract)
                            nc.vector.tensor_tensor(dy[:, sl], ry[:, sl],
                                                    CB("qt2b"), OP.mult)
                    else:
                        nc.vector.tensor_tensor(
                            ey[:].rearrange("p (b w) -> p b w", b=2),
                            ps[:].rearrange("p (b w) -> p b w", b=2),
                            rqt2.unsqueeze(1)
                            .broadcast_to([128, 2, 512]), OP.mult)
                        nc.vector.tensor_scalar(ry[:], ey[:], C_ROUND,
                                                C_ROUND, OP.add, OP.subtract)
                        nc.vector.tensor_tensor(
                            dy[:].rearrange("p (b w) -> p b w", b=2),
                            ry[:].rearrange("p (b w) -> p b w", b=2),
                            CB("qt2b").unsqueeze(1)
                            .broadcast_to([128, 2, 512]), OP.mult)
                    decy.append(dy)

                psc = ptp.tile([128, 1024], F32, tag="tp", name="pstp")
                for s in range(4):
                    nc.tensor.matmul(
                        psc[:, 512 * (s // 2):512 * (s // 2) + 512],
                        CW("w2c_hi" if s % 2 else "w2c_lo"),
                        t1c[s // 2][:, 512 * (s % 2):512 * (s % 2) + 512],
                        start=(s % 2 == 0), stop=(s % 2 == 1))
                ec = qp.tile([128, 1024], F32, tag="ey", name=f"ec_{img}")
                rc = qp.tile([128, 1024], BF16, tag="ry", name=f"rc_{img}")
                decc = dp.tile([128, 1024], BF16, tag="decc",
                               name=f"decc_{img}")
                if img == IMG_PER_CORE - 1:
                    for h in range(2):
                        sl = slice(512 * h, 512 * (h + 1))
                        nc.vector.tensor_tensor(ec[:, sl], psc[:, sl],
                                                rqt2, OP.mult)
                        nc.vector.tensor_scalar(rc[:, sl], ec[:, sl],
                                                C_ROUND, C_ROUND,
                                                OP.add, OP.subtract)
                        nc.vector.tensor_tensor(decc[:, sl], rc[:, sl],
                                                CB("qt2b"), OP.mult)
                else:
                    nc.vector.tensor_tensor(
                        ec[:].rearrange("p (b w) -> p b w", b=2),
                        psc[:].rearrange("p (b w) -> p b w", b=2),
                        rqt2.unsqueeze(1)
                        .broadcast_to([128, 2, 512]), OP.mult)
                    nc.vector.tensor_scalar(rc[:], ec[:], C_ROUND, C_ROUND,
                                            OP.add, OP.subtract)
                    nc.vector.tensor_tensor(
                        decc[:].rearrange("p (b w) -> p b w", b=2),
                        rc[:].rearrange("p (b w) -> p b w", b=2),
                        CB("qt2b").unsqueeze(1)
                        .broadcast_to([128, 2, 512]), OP.mult)

                # ---- S3: fused W-IDCT + transpose (bf16 matmuls) ----
                t2y, t2c = [], []
                for v in range(2):
                    ps = pmm.tile([128, 1024], F32, tag="mm", name="psmm")
                    for b in range(2):
                        t = 2 * v + b
                        for s in range(4):
                            nc.tensor.matmul(
                                ps[:, 512 * b + 128 * s:512 * b + 128 * (s + 1)],
                                decy[s // 2][:, 512 * (s % 2) + 128 * t:
                                             512 * (s % 2) + 128 * (t + 1)],
                                CB("bdw_b"), start=True, stop=True)
                    sy = dp.tile([128, 1024], BF16, tag="t2y",
                                 name=f"t2y_{img}_{v}")
                    nc.scalar.activation(sy[:], ps[:], ACT.Identity,
                                         bias=lpos)
                    t2y.append(sy)
                for v in range(2):
                    ps = pmm.tile([128, 1024], F32, tag="mm", name="psmm")
                    for b in range(2):
                        t = 2 * v + b
                        for g in range(2):
                            nc.tensor.matmul(
                                ps[:, 512 * b + 256 * g:512 * b + 256 * (g + 1)],
                                decc[:, 512 * g + 128 * t:
                                     512 * g + 128 * (t + 1)],
                                CB("pud2"), start=True, stop=True)
                    sc = dp.tile([128, 1024], BF16, tag="t2c",
                                 name=f"t2c_{img}_{v}")
                    nc.scalar.activation(sc[:], ps[:], ACT.Copy)
                    t2c.append(sc)

                # ---- P4: H-IDCT + color + clamp + store ----
                for ci, cname in enumerate(("R", "G", "B")):
                    for v in range(2):
                        if img == IMG_PER_CORE - 1 and (2 * ci + v) % 2:
                            ps = ptp.tile([128, 1024], F32, tag="tp",
                                          name="pstp")
                        else:
                            ps = pmm.tile([128, 1024], F32, tag="mm",
                                          name="psmm")
                        for b in range(2):
                            nc.tensor.matmul(
                                ps[:, 512 * b:512 * (b + 1)], CB("w4y_b"),
                                t2y[v][:, 512 * b:512 * (b + 1)],
                                start=True, stop=False)
                            nc.tensor.matmul(
                                ps[:, 512 * b:512 * (b + 1)],
                                CB(f"w4c{cname}_b"),
                                t2c[v][:, 512 * b:512 * (b + 1)],
                                start=False, stop=True)
                        og = op_.tile([128, 1024], BF16, tag="og",
                                      name=f"og_{img}_{ci}_{v}")
                        if img == IMG_PER_CORE - 1:
                            # last image: clamp+store per half; halves
                            # alternate scalar/sync rings so the tail
                            # drains on two queues
                            for b in range(2):
                                sl = slice(512 * b, 512 * (b + 1))
                                nc.vector.tensor_scalar(og[:, sl], ps[:, sl],
                                                        0.0, 1.0,
                                                        OP.max, OP.min)
                                t = 2 * v + b
                                eng = nc.gpsimd if b == 0 else nc.sync
                                eng.dma_start(
                                    out_d[img, ci, 128 * t:128 * (t + 1), :],
                                    og[:, sl])
                        else:
                            nc.vector.tensor_scalar(og[:], ps[:], 0.0, 1.0,
                                                    OP.max, OP.min)
                            nc.gpsimd.dma_start(
                                out_d[img, ci, 256 * v:256 * (v + 1), :]
                                .rearrange("(b p) w -> p b w", b=2),
                                og[:].rearrange("p (b w) -> p b w", b=2))
    nc.compile()
    return nc


_NC_CACHE = None


def kernel(input, quantize):
    global _NC_CACHE
    input = np.asarray(input, dtype=np.float32)
    quantize = np.asarray(quantize, dtype=np.float32)
    consts = _build_consts(quantize)
    if _NC_CACHE is None:
        _NC_CACHE = _build_nc()
    nc = _NC_CACHE

    in_maps = []
    for core in range(N_CORES):
        shard = np.ascontiguousarray(
            input[core * IMG_PER_CORE:(core + 1) * IMG_PER_CORE])
        m = {"x": shard}
        m.update(consts)
        in_maps.append(m)
    trace = bool(os.environ.get("JPEG_TRACE"))
    kw = {}
    if trace:
        kw["trace"] = True
        td = os.environ.get("JPEG_TRACE_DIR")
        if td:
            os.makedirs(td, exist_ok=True)
            kw["tmpdir"] = td
    res = bass_utils.run_bass_kernel_spmd(nc, in_maps,
                                          core_ids=list(range(N_CORES)), **kw)
    global LAST_RESULT
    LAST_RESULT = res
    out = np.concatenate(
        [np.asarray(res.results[i]["out"]) for i in range(N_CORES)], axis=0)
    return out.astype(np.float32)


LAST_RESULT = None


# revision 12
# speedup vs baseline: 1.2258x; 1.0703x over previous
"""JPEG layer (nn_JpegLayer) Trainium2 Bass kernel, 8-core data parallel.

Pipeline per image (per core: 4 images of [3,512,512]):
  P1: 3-accum f32r matmuls fold RGB->YCC color mix + H-DCT (+ vertical
      2x-pool for chroma) into [128,1024] 2-bank PSUM pairs. The Y drain
      (ACT Identity) subtracts sqrt(8)*L on h-freq DC rows = the -L level
      shift folded through the H-DCT.
  T1: PE transposes (f32r, identity rhs) -> [w, h-freq] pairs.
  P2: W-DCT (f32r). Chroma's 4 M=64 outputs pack into one [128,1024]
      pair via zero-padded [128,128] weight halves accumulated into the
      same region (the ISA rejects nonzero PSUM dst partition offsets).
  Q : all on DVE over [128,1024] pairs: e = d*(1/q) (TT, psum read,
      [128,512] table broadcast via stride-0 AP); r = (e + 1.5*2^23) -
      1.5*2^23 (dual-op tensor_scalar, bf16 out -- |r| < 256 so bf16 is
      exact); dec = r*q (bf16 TT).
  S3: fused W-IDCT + transpose as regular bf16 matmuls with dec chunks
      as the stationary operand (replaces P3 matmuls + T2 transposes).
  P4: bf16 N=1024 matmuls: H-IDCT + YCC->RGB fold (+ v-upsample chroma),
      one Y + one C matmul accumulated per [128,1024] psum tile.
  out: clamp [0,1] via DVE dual-op tensor_scalar, bf16 store on the
      scalar DGE ring (loads ride the sync ring), host upcasts to f32.

v2 scheduling fixes (baseline 121.1us):
  - consts packed into 3 dram tensors (f32r/bf16/f32) -> 3 DMAs, not 19.
  - input loads batched per (img,t): one [128, 3*512] DMA spanning all
    3 channels; img 0 spreads t across sync/scalar/gpsimd rings so both
    P1 j=0 tiles land in parallel.
  - PE warm-up: 8 throwaway N=512 matmuls on the const tile right after
    its DMA lands. The PE HAM clock gate defaults to K=4/8 (1.2 GHz) and
    only releases after ~3.4us of sustained activity; the baseline ran
    the whole first image's P1 at half clock (HAM warm only at 29.4us).
    Warming during the DMA head makes real work start at 2.4 GHz.
  - stores go on the gpsimd ring (sync keeps loads, scalar only consts +
    ACT drains), last image's store halves alternate gpsimd/sync.
"""
import os
import sys
sys.path.insert(0, '/opt/trn_rl_repo')
import numpy as np
import ml_dtypes
import concourse.bacc as bacc
import concourse.bass as bass
import concourse.mybir as mybir
import concourse.tile as tile
from concourse import bass_utils

N_CORES = 8
IMG_PER_CORE = 4
H = W = 512
LEVEL = np.float32(128.0 / 255.0)
C_ROUND = 12582912.0   # 1.5*2^23: (x+C)-C == round-half-even(x)
F32 = mybir.dt.float32
F32R = mybir.dt.float32r
BF16 = mybir.dt.bfloat16

RGB2YCC = np.array([[0.299, 0.587, 0.114],
                    [-0.168735892, -0.331264108, 0.5],
                    [0.5, -0.418687589, -0.081312411]], dtype=np.float32)
CB_C = np.array([0.0, -0.344136286, 1.772], dtype=np.float32)
CR_C = np.array([1.402, -0.714136286, 0.0], dtype=np.float32)

# offsets (in 128-col units) into the packed f32r const tiles: cfa holds
# the P1 weights (start-critical), cfb the T1/P2 weights
OFA = {"w1y0": 0, "w1y1": 1, "w1y2": 2, "w1c0": 3, "w1c1": 4, "w1c2": 5}
OFB = {"ident": 0, "w2y": 1, "w2c_lo": 2, "w2c_hi": 3}
NFA = 6 * 128
NFB = 4 * 128
# offsets into the packed bf16 const tile
OB = {"qt2b": (0, 512), "bdw_b": (512, 128), "pud2": (640, 256),
      "w4y_b": (896, 128), "w4cR_b": (1024, 128), "w4cG_b": (1152, 128),
      "w4cB_b": (1280, 128)}
NBF = 1408
# f32 tile: rqt2 [0:512], lneg [512:513], lpos [513:514]
NF32 = 514


def _dct8():
    i = np.arange(8)[:, None].astype(np.float64)
    j = np.arange(8)[None, :].astype(np.float64)
    m = np.sqrt(2.0 / 8) * np.cos(np.pi * (2 * j + 1) * i / 16.0)
    m[0, :] = 1.0 / np.sqrt(8.0)
    return m.astype(np.float32)


def _blockdiag(b, reps):
    r, c = b.shape
    out = np.zeros((r * reps, c * reps), dtype=np.float32)
    for k in range(reps):
        out[k * r:(k + 1) * r, k * c:(k + 1) * c] = b
    return out


def _build_consts(quantize):
    D = _dct8()
    BD_T = _blockdiag(D.T, 16)             # [128,128] fwd 1D-DCT as lhsT
    BD = _blockdiag(D, 16)                 # [128,128] inverse
    pf8 = np.zeros((16, 8), dtype=np.float32)
    for ii in range(8):
        for dh in range(2):
            pf8[2 * ii + dh, :] = D[:, ii] * 0.5
    PF = _blockdiag(pf8, 8)                # [128, 64]
    pu8 = np.zeros((8, 16), dtype=np.float32)
    for jj in range(8):
        for dw in range(2):
            pu8[:, 2 * jj + dw] = D[:, jj]
    PU = _blockdiag(pu8, 8)                # [64, 128]

    bf = ml_dtypes.bfloat16

    fa = np.zeros((128, NFA), dtype=np.float32)
    fb = np.zeros((128, NFB), dtype=np.float32)
    def putf(name, arr):
        if name in OFA:
            fa[:, OFA[name] * 128:OFA[name] * 128 + arr.shape[1]] = arr
        else:
            fb[:, OFB[name] * 128:OFB[name] * 128 + arr.shape[1]] = arr
    for c in range(3):
        putf(f"w1y{c}", RGB2YCC[0, c] * BD_T)
        putf(f"w1c{c}", np.concatenate(
            [RGB2YCC[1, c] * PF, RGB2YCC[2, c] * PF], axis=1))
    putf("ident", np.eye(128, dtype=np.float32))
    putf("w2y", BD_T)
    w2c_lo = np.zeros((128, 128), dtype=np.float32)
    w2c_lo[:, 0:64] = PF
    w2c_hi = np.zeros((128, 128), dtype=np.float32)
    w2c_hi[:, 64:128] = PF
    putf("w2c_lo", w2c_lo)
    putf("w2c_hi", w2c_hi)

    q = (np.round(quantize[0].astype(np.float32) * np.float32(255.0))
         / np.float32(255.0)).astype(np.float32)
    rq = (1.0 / q.astype(np.float64)).astype(np.float32)

    bfc = np.zeros((128, NBF), dtype=np.float32)
    def putb(name, arr):
        o, n = OB[name]
        assert arr.shape[1] == n
        bfc[:, o:o + n] = arr
    putb("qt2b", np.tile(q.T, (16, 64)))
    putb("bdw_b", BD)
    pud2 = np.zeros((128, 256), dtype=np.float32)
    pud2[0:64, 0:128] = PU
    pud2[64:128, 128:256] = PU
    putb("pud2", pud2)
    putb("w4y_b", BD)
    for name, cb, cr in (("R", CB_C[0], CR_C[0]), ("G", CB_C[1], CR_C[1]),
                         ("B", CB_C[2], CR_C[2])):
        m = np.zeros((128, 128), dtype=np.float32)
        m[0:64, :] = cb * PU
        m[64:128, :] = cr * PU
        putb(f"w4c{name}_b", m)

    f32c = np.zeros((128, NF32), dtype=np.float32)
    f32c[:, 0:512] = np.tile(rq.T, (16, 64))
    f32c[0::8, 512] = -np.float32(np.sqrt(8.0) * LEVEL)
    f32c[0::8, 513] = np.float32(np.sqrt(8.0) * LEVEL)

    return {"cfa": fa, "cfb": fb, "cbf": bfc.astype(bf), "cf32": f32c}


def _build_nc():
    nc = bacc.Bacc("TRN2", target_bir_lowering=False, debug=False,
                   enable_asserts=False, num_devices=N_CORES)
    x_d = nc.dram_tensor("x", [IMG_PER_CORE, 3, H, W], F32R,
                         kind="ExternalInput").ap()
    out_d = nc.dram_tensor("out", [IMG_PER_CORE, 3, H, W], BF16,
                           kind="ExternalOutput").ap()
    cfa_d = nc.dram_tensor("cfa", [128, NFA], F32R,
                           kind="ExternalInput").ap()
    cfb_d = nc.dram_tensor("cfb", [128, NFB], F32R,
                           kind="ExternalInput").ap()
    cbf_d = nc.dram_tensor("cbf", [128, NBF], BF16,
                           kind="ExternalInput").ap()
    cf32_d = nc.dram_tensor("cf32", [128, NF32], F32,
                            kind="ExternalInput").ap()

    ACT = mybir.ActivationFunctionType
    OP = mybir.AluOpType

    with tile.TileContext(nc) as tc:
        with tc.tile_pool(name="consts", bufs=1) as cp, \
             tc.tile_pool(name="xin", bufs=10) as xp, \
             tc.tile_pool(name="fwd", bufs=4) as fp, \
             tc.tile_pool(name="qnt", bufs=4) as qp, \
             tc.tile_pool(name="dcd", bufs=3) as dp, \
             tc.tile_pool(name="outp", bufs=8) as op_, \
             tc.tile_pool(name="psmm", bufs=2, space="PSUM") as pmm, \
             tc.tile_pool(name="pstp", bufs=2, space="PSUM") as ptp:

            # ---- packed const loads, all on the scalar ring in order of
            # first use: P1 weights, T1/P2 weights, q tables, bf16 weights
            cfa = cp.tile([128, NFA], F32R, tag="cfa", name="cfa")
            nc.scalar.dma_start(cfa[:], cfa_d)
            cfb = cp.tile([128, NFB], F32R, tag="cfb", name="cfb")
            nc.scalar.dma_start(cfb[:], cfb_d)
            cf32 = cp.tile([128, NF32], F32, tag="cf32", name="cf32")
            nc.scalar.dma_start(cf32[:], cf32_d)
            cbf = cp.tile([128, NBF], BF16, tag="cbf", name="cbf")
            nc.scalar.dma_start(cbf[:], cbf_d)

            def CW(name):  # [128,128] f32r const slice
                if name in OFA:
                    return cfa[:, OFA[name] * 128:OFA[name] * 128 + 128]
                return cfb[:, OFB[name] * 128:OFB[name] * 128 + 128]

            def CB(name):  # bf16 const slice
                o, n = OB[name]
                return cbf[:, o:o + n]

            rqt2 = cf32[:, 0:512]
            lneg = cf32[:, 512:513]
            lpos = cf32[:, 513:514]

            # ---- PE warm-up: release the HAM clock gate during the DMA
            # head. The PE defaults to K=4/8 (1.2 GHz) and only unthrottles
            # after ~3.4us of sustained activity; throwaway N=512 matmuls
            # over a memset tile (no DMA dependency -- PE is busy right out
            # of the framework preamble) flip it before real work arrives.
            junk = cp.tile([128, 512], BF16, tag="junk", name="junk")
            nc.gpsimd.memset(junk[:], 0.0)
            warm = ptp.tile([128, 1024], F32, tag="tp", name="pstp")
            for k in range(9):
                nc.tensor.matmul(warm[:, 0:512], junk[:, 0:128],
                                 junk[:], start=True, stop=True)

            for img in range(IMG_PER_CORE):
                # ---- batched RGB loads: one DMA per (img, t) spanning all
                # 3 channels; img 0 fans out across 3 DGE rings ----
                X = {}
                for t in range(4):
                    if img == 0 and t < 2:
                        # per-channel pieces so P1 j=0 can start on the
                        # first [128,512] to land; t0 on sync, t1 gpsimd
                        eng = nc.sync if t == 0 else nc.gpsimd
                        xt = xp.tile([128, 1536], F32R, tag="x",
                                     name=f"x_{img}_{t}")
                        for c in range(3):
                            eng.dma_start(xt[:, 512 * c:512 * (c + 1)],
                                          x_d[img, c,
                                              128 * t:128 * (t + 1), :])
                            X[c, t] = xt[:, 512 * c:512 * (c + 1)]
                        continue
                    xt = xp.tile([128, 1536], F32R, tag="x",
                                 name=f"x_{img}_{t}")
                    if img == 0:
                        eng = nc.sync if t == 2 else nc.gpsimd
                    else:
                        eng = nc.sync
                    src = x_d[img].rearrange("c (t p) w -> t p c w",
                                             p=128)[t]
                    eng.dma_start(
                        xt[:].rearrange("p (c w) -> p c w", c=3), src)
                    for c in range(3):
                        X[c, t] = xt[:, 512 * c:512 * (c + 1)]

                # ---- P1: color + H-DCT (+v-pool chroma), pairs over t ----
                d1y, d1c = [], []
                for j in range(2):
                    psY = ptp.tile([128, 1024], F32, tag="tp", name="pstp")
                    for b in range(2):
                        t = 2 * j + b
                        for c in range(3):
                            nc.tensor.matmul(psY[:, 512 * b:512 * (b + 1)],
                                             CW(f"w1y{c}"), X[c, t],
                                             start=(c == 0), stop=(c == 2))
                    ty = fp.tile([128, 1024], F32R, tag="d1y",
                                 name=f"d1y_{img}_{j}")
                    nc.scalar.activation(ty[:], psY[:], ACT.Identity,
                                         bias=lneg)
                    d1y.append(ty)
                    psC = ptp.tile([128, 1024], F32, tag="tp", name="pstp")
                    for b in range(2):
                        t = 2 * j + b
                        for c in range(3):
                            nc.tensor.matmul(psC[:, 512 * b:512 * (b + 1)],
                                             CW(f"w1c{c}"), X[c, t],
                                             start=(c == 0), stop=(c == 2))
                    tcc = fp.tile([128, 1024], F32R, tag="d1c",
                                  name=f"d1c_{img}_{j}")
                    nc.scalar.activation(tcc[:], psC[:], ACT.Copy)
                    d1c.append(tcc)

                # ---- T1: PE transposes, pairs over s ----
                t1y, t1c = [], []
                for u in range(2):
                    pty = ptp.tile([128, 1024], F32R, tag="tp", name="pstp")
                    for b in range(2):
                        s = 2 * u + b
                        for t in range(4):
                            nc.tensor.transpose(
                                pty[:, 512 * b + 128 * t:512 * b + 128 * (t + 1)],
                                d1y[t // 2][:, 512 * (t % 2) + 128 * s:
                                            512 * (t % 2) + 128 * (s + 1)],
                                CW("ident"))
                    sy = fp.tile([128, 1024], F32R, tag="t1y",
                                 name=f"t1y_{img}_{u}")
                    nc.scalar.activation(sy[:], pty[:], ACT.Copy)
                    t1y.append(sy)
                for u in range(2):
                    ptc = ptp.tile([128, 1024], F32R, tag="tp", name="pstp")
                    for b in range(2):
                        s = 2 * u + b
                        for t in range(4):
                            nc.tensor.transpose(
                                ptc[:, 512 * b + 128 * t:512 * b + 128 * (t + 1)],
                                d1c[t // 2][:, 512 * (t % 2) + 128 * s:
                                            512 * (t % 2) + 128 * (s + 1)],
                                CW("ident"))
                    sc = fp.tile([128, 1024], F32R, tag="t1c",
                                 name=f"t1c_{img}_{u}")
                    nc.scalar.activation(sc[:], ptc[:], ACT.Copy)
                    t1c.append(sc)

                # ---- P2 + quantize (all DVE) ----
                decy = []
                for u in range(2):
                    ps = pmm.tile([128, 1024], F32, tag="mm", name="psmm")
                    for b in range(2):
                        nc.tensor.matmul(ps[:, 512 * b:512 * (b + 1)],
                                         CW("w2y"),
                                         t1y[u][:, 512 * b:512 * (b + 1)],
                                         start=True, stop=True)
                    ey = qp.tile([128, 1024], F32, tag="ey",
                                 name=f"ey_{img}_{u}")
                    ry = qp.tile([128, 1024], BF16, tag="ry",
                                 name=f"ry_{img}_{u}")
                    dy = dp.tile([128, 1024], BF16, tag="decy",
                                 name=f"decy_{img}_{u}")
                    if img == IMG_PER_CORE - 1:
                        for h in range(2):
                            sl = slice(512 * h, 512 * (h + 1))
                            nc.vector.tensor_tensor(ey[:, sl], ps[:, sl],
                                                    rqt2, OP.mult)
                            nc.vector.tensor_scalar(ry[:, sl], ey[:, sl],
                                                    C_ROUND, C_ROUND,
                                                    OP.add, OP.subtract)
                            nc.vector.tensor_tensor(dy[:, sl], ry[:, sl],
                                                    CB("qt2b"), OP.mult)
                    else:
                        nc.vector.tensor_tensor(
                            ey[:].rearrange("p (b w) -> p b w", b=2),
                            ps[:].rearrange("p (b w) -> p b w", b=2),
                            rqt2.unsqueeze(1)
                            .broadcast_to([128, 2, 512]), OP.mult)
                        nc.vector.tensor_scalar(ry[:], ey[:], C_ROUND,
                                                C_ROUND, OP.add, OP.subtract)
                        nc.vector.tensor_tensor(
                            dy[:].rearrange("p (b w) -> p b w", b=2),
                            ry[:].rearrange("p (b w) -> p b w", b=2),
                            CB("qt2b").unsqueeze(1)
                            .broadcast_to([128, 2, 512]), OP.mult)
                    decy.append(dy)

                psc = pmm.tile([128, 1024], F32, tag="mm", name="psmm")
                for s in range(4):
                    nc.tensor.matmul(
                        psc[:, 512 * (s // 2):512 * (s // 2) + 512],
                        CW("w2c_hi" if s % 2 else "w2c_lo"),
                        t1c[s // 2][:, 512 * (s % 2):512 * (s % 2) + 512],
                        start=(s % 2 == 0), stop=(s % 2 == 1))
                ec = qp.tile([128, 1024], F32, tag="ey", name=f"ec_{img}")
                rc = qp.tile([128, 1024], BF16, tag="ry", name=f"rc_{img}")
                decc = dp.tile([128, 1024], BF16, tag="decc",
                               name=f"decc_{img}")
                if img == IMG_PER_CORE - 1:
                    for h in range(2):
                        sl = slice(512 * h, 512 * (h + 1))
                        nc.vector.tensor_tensor(ec[:, sl], psc[:, sl],
                                                rqt2, OP.mult)
                        nc.vector.tensor_scalar(rc[:, sl], ec[:, sl],
                                                C_ROUND, C_ROUND,
                                                OP.add, OP.subtract)
                        nc.vector.tensor_tensor(decc[:, sl], rc[:, sl],
                                                CB("qt2b"), OP.mult)
                else:
                    nc.vector.tensor_tensor(
                        ec[:].rearrange("p (b w) -> p b w", b=2),
                        psc[:].rearrange("p (b w) -> p b w", b=2),
                        rqt2.unsqueeze(1)
                        .broadcast_to([128, 2, 512]), OP.mult)
                    nc.vector.tensor_scalar(rc[:], ec[:], C_ROUND, C_ROUND,
                                            OP.add, OP.subtract)
                    nc.vector.tensor_tensor(
                        decc[:].rearrange("p (b w) -> p b w", b=2),
                        rc[:].rearrange("p (b w) -> p b w", b=2),
                        CB("qt2b").unsqueeze(1)
                        .broadcast_to([128, 2, 512]), OP.mult)

                # ---- S3: fused W-IDCT + transpose (bf16 matmuls) ----
                t2y, t2c = [], []
                for v in range(2):
                    ps = pmm.tile([128, 1024], F32, tag="mm", name="psmm")
                    for b in range(2):
                        t = 2 * v + b
                        for s in range(4):
                            nc.tensor.matmul(
                                ps[:, 512 * b + 128 * s:512 * b + 128 * (s + 1)],
                                decy[s // 2][:, 512 * (s % 2) + 128 * t:
                                             512 * (s % 2) + 128 * (t + 1)],
                                CB("bdw_b"), start=True, stop=True)
                    sy = dp.tile([128, 1024], BF16, tag="t2y",
                                 name=f"t2y_{img}_{v}")
                    nc.scalar.activation(sy[:], ps[:], ACT.Identity,
                                         bias=lpos)
                    t2y.append(sy)
                for v in range(2):
                    ps = pmm.tile([128, 1024], F32, tag="mm", name="psmm")
                    for b in range(2):
                        t = 2 * v + b
                        for g in range(2):
                            nc.tensor.matmul(
                                ps[:, 512 * b + 256 * g:512 * b + 256 * (g + 1)],
                                decc[:, 512 * g + 128 * t:
                                     512 * g + 128 * (t + 1)],
                                CB("pud2"), start=True, stop=True)
                    sc = dp.tile([128, 1024], BF16, tag="t2c",
                                 name=f"t2c_{img}_{v}")
                    nc.scalar.activation(sc[:], ps[:], ACT.Copy)
                    t2c.append(sc)

                # ---- P4: H-IDCT + color + clamp + store ----
                for ci, cname in enumerate(("R", "G", "B")):
                    for v in range(2):
                        if img == IMG_PER_CORE - 1 and (2 * ci + v) % 2:
                            ps = ptp.tile([128, 1024], F32, tag="tp",
                                          name="pstp")
                        else:
                            ps = pmm.tile([128, 1024], F32, tag="mm",
                                          name="psmm")
                        for b in range(2):
                            nc.tensor.matmul(
                                ps[:, 512 * b:512 * (b + 1)], CB("w4y_b"),
                                t2y[v][:, 512 * b:512 * (b + 1)],
                                start=True, stop=False)
                            nc.tensor.matmul(
                                ps[:, 512 * b:512 * (b + 1)],
                                CB(f"w4c{cname}_b"),
                                t2c[v][:, 512 * b:512 * (b + 1)],
                                start=False, stop=True)
                        og = op_.tile([128, 1024], BF16, tag="og",
                                      name=f"og_{img}_{ci}_{v}")
                        if img == IMG_PER_CORE - 1:
                            # last image: clamp+store per half; halves
                            # alternate scalar/sync rings so the tail
                            # drains on two queues
                            for b in range(2):
                                sl = slice(512 * b, 512 * (b + 1))
                                nc.vector.tensor_scalar(og[:, sl], ps[:, sl],
                                                        0.0, 1.0,
                                                        OP.max, OP.min)
                                t = 2 * v + b
                                eng = nc.gpsimd if b == 0 else nc.sync
                                eng.dma_start(
                                    out_d[img, ci, 128 * t:128 * (t + 1), :],
                                    og[:, sl])
                        else:
                            nc.vector.tensor_scalar(og[:], ps[:], 0.0, 1.0,
                                                    OP.max, OP.min)
                            nc.gpsimd.dma_start(
                                out_d[img, ci, 256 * v:256 * (v + 1), :]
                                .rearrange("(b p) w -> p b w", b=2),
                                og[:].rearrange("p (b w) -> p b w", b=2))
    nc.compile()
    return nc


_NC_CACHE = None


def kernel(input, quantize):
    global _NC_CACHE
    input = np.asarray(input, dtype=np.float32)
    quantize = np.asarray(quantize, dtype=np.float32)
    consts = _build_consts(quantize)
    if _NC_CACHE is None:
        _NC_CACHE = _build_nc()
    nc = _NC_CACHE

    in_maps = []
    for core in range(N_CORES):
        shard = np.ascontiguousarray(
            input[core * IMG_PER_CORE:(core + 1) * IMG_PER_CORE])
        m = {"x": shard}
        m.update(consts)
        in_maps.append(m)
    trace = bool(os.environ.get("JPEG_TRACE"))
    kw = {}
    if trace:
        kw["trace"] = True
        td = os.environ.get("JPEG_TRACE_DIR")
        if td:
            os.makedirs(td, exist_ok=True)
            kw["tmpdir"] = td
    res = bass_utils.run_bass_kernel_spmd(nc, in_maps,
                                          core_ids=list(range(N_CORES)), **kw)
    global LAST_RESULT
    LAST_RESULT = res
    out = np.concatenate(
        [np.asarray(res.results[i]["out"]) for i in range(N_CORES)], axis=0)
    return out.astype(np.float32)


LAST_RESULT = None


# revision 13
# speedup vs baseline: 1.2323x; 1.0053x over previous
"""JPEG layer (nn_JpegLayer) Trainium2 Bass kernel, 8-core data parallel.

Pipeline per image (per core: 4 images of [3,512,512]):
  P1: 3-accum f32r matmuls fold RGB->YCC color mix + H-DCT (+ vertical
      2x-pool for chroma) into [128,1024] 2-bank PSUM pairs. The Y drain
      (ACT Identity) subtracts sqrt(8)*L on h-freq DC rows = the -L level
      shift folded through the H-DCT.
  T1: PE transposes (f32r, identity rhs) -> [w, h-freq] pairs.
  P2: W-DCT (f32r). Chroma's 4 M=64 outputs pack into one [128,1024]
      pair via zero-padded [128,128] weight halves accumulated into the
      same region (the ISA rejects nonzero PSUM dst partition offsets).
  Q : all on DVE over [128,1024] pairs: e = d*(1/q) (TT, psum read,
      [128,512] table broadcast via stride-0 AP); r = (e + 1.5*2^23) -
      1.5*2^23 (dual-op tensor_scalar, bf16 out -- |r| < 256 so bf16 is
      exact); dec = r*q (bf16 TT).
  S3: fused W-IDCT + transpose as regular bf16 matmuls with dec chunks
      as the stationary operand (replaces P3 matmuls + T2 transposes).
  P4: bf16 N=1024 matmuls: H-IDCT + YCC->RGB fold (+ v-upsample chroma),
      one Y + one C matmul accumulated per [128,1024] psum tile.
  out: clamp [0,1] via DVE dual-op tensor_scalar, bf16 store on the
      scalar DGE ring (loads ride the sync ring), host upcasts to f32.

v2 scheduling fixes (baseline 121.1us):
  - consts packed into 3 dram tensors (f32r/bf16/f32) -> 3 DMAs, not 19.
  - input loads batched per (img,t): one [128, 3*512] DMA spanning all
    3 channels; img 0 spreads t across sync/scalar/gpsimd rings so both
    P1 j=0 tiles land in parallel.
  - PE warm-up: 8 throwaway N=512 matmuls on the const tile right after
    its DMA lands. The PE HAM clock gate defaults to K=4/8 (1.2 GHz) and
    only releases after ~3.4us of sustained activity; the baseline ran
    the whole first image's P1 at half clock (HAM warm only at 29.4us).
    Warming during the DMA head makes real work start at 2.4 GHz.
  - stores go on the gpsimd ring (sync keeps loads, scalar only consts +
    ACT drains), last image's store halves alternate gpsimd/sync.
"""
import os
import sys
sys.path.insert(0, '/opt/trn_rl_repo')
import numpy as np
import ml_dtypes
import concourse.bacc as bacc
import concourse.bass as bass
import concourse.mybir as mybir
import concourse.tile as tile
from concourse import bass_utils

N_CORES = 8
IMG_PER_CORE = 4
H = W = 512
LEVEL = np.float32(128.0 / 255.0)
C_ROUND = 12582912.0   # 1.5*2^23: (x+C)-C == round-half-even(x)
F32 = mybir.dt.float32
F32R = mybir.dt.float32r
BF16 = mybir.dt.bfloat16

RGB2YCC = np.array([[0.299, 0.587, 0.114],
                    [-0.168735892, -0.331264108, 0.5],
                    [0.5, -0.418687589, -0.081312411]], dtype=np.float32)
CB_C = np.array([0.0, -0.344136286, 1.772], dtype=np.float32)
CR_C = np.array([1.402, -0.714136286, 0.0], dtype=np.float32)

# offsets (in 128-col units) into the packed f32r const tiles: cfa holds
# the P1 weights (start-critical), cfb the T1/P2 weights
OFA = {"w1y0": 0, "w1y1": 1, "w1y2": 2, "w1c0": 3, "w1c1": 4, "w1c2": 5}
OFB = {"ident": 0, "w2y": 1, "w2c_lo": 2, "w2c_hi": 3}
NFA = 6 * 128
NFB = 4 * 128
# offsets into the packed bf16 const tile
OB = {"qt2b": (0, 512), "bdw_b": (512, 128), "pud2": (640, 256),
      "w4y_b": (896, 128), "w4cR_b": (1024, 128), "w4cG_b": (1152, 128),
      "w4cB_b": (1280, 128)}
NBF = 1408
# f32 tile: rqt2 [0:512], lneg [512:513], lpos [513:514]
NF32 = 514


def _dct8():
    i = np.arange(8)[:, None].astype(np.float64)
    j = np.arange(8)[None, :].astype(np.float64)
    m = np.sqrt(2.0 / 8) * np.cos(np.pi * (2 * j + 1) * i / 16.0)
    m[0, :] = 1.0 / np.sqrt(8.0)
    return m.astype(np.float32)


def _blockdiag(b, reps):
    r, c = b.shape
    out = np.zeros((r * reps, c * reps), dtype=np.float32)
    for k in range(reps):
        out[k * r:(k + 1) * r, k * c:(k + 1) * c] = b
    return out


def _build_consts(quantize):
    D = _dct8()
    BD_T = _blockdiag(D.T, 16)             # [128,128] fwd 1D-DCT as lhsT
    BD = _blockdiag(D, 16)                 # [128,128] inverse
    pf8 = np.zeros((16, 8), dtype=np.float32)
    for ii in range(8):
        for dh in range(2):
            pf8[2 * ii + dh, :] = D[:, ii] * 0.5
    PF = _blockdiag(pf8, 8)                # [128, 64]
    pu8 = np.zeros((8, 16), dtype=np.float32)
    for jj in range(8):
        for dw in range(2):
            pu8[:, 2 * jj + dw] = D[:, jj]
    PU = _blockdiag(pu8, 8)                # [64, 128]

    bf = ml_dtypes.bfloat16

    fa = np.zeros((128, NFA), dtype=np.float32)
    fb = np.zeros((128, NFB), dtype=np.float32)
    def putf(name, arr):
        if name in OFA:
            fa[:, OFA[name] * 128:OFA[name] * 128 + arr.shape[1]] = arr
        else:
            fb[:, OFB[name] * 128:OFB[name] * 128 + arr.shape[1]] = arr
    for c in range(3):
        putf(f"w1y{c}", RGB2YCC[0, c] * BD_T)
        putf(f"w1c{c}", np.concatenate(
            [RGB2YCC[1, c] * PF, RGB2YCC[2, c] * PF], axis=1))
    putf("ident", np.eye(128, dtype=np.float32))
    putf("w2y", BD_T)
    w2c_lo = np.zeros((128, 128), dtype=np.float32)
    w2c_lo[:, 0:64] = PF
    w2c_hi = np.zeros((128, 128), dtype=np.float32)
    w2c_hi[:, 64:128] = PF
    putf("w2c_lo", w2c_lo)
    putf("w2c_hi", w2c_hi)

    q = (np.round(quantize[0].astype(np.float32) * np.float32(255.0))
         / np.float32(255.0)).astype(np.float32)
    rq = (1.0 / q.astype(np.float64)).astype(np.float32)

    bfc = np.zeros((128, NBF), dtype=np.float32)
    def putb(name, arr):
        o, n = OB[name]
        assert arr.shape[1] == n
        bfc[:, o:o + n] = arr
    putb("qt2b", np.tile(q.T, (16, 64)))
    putb("bdw_b", BD)
    pud2 = np.zeros((128, 256), dtype=np.float32)
    pud2[0:64, 0:128] = PU
    pud2[64:128, 128:256] = PU
    putb("pud2", pud2)
    putb("w4y_b", BD)
    for name, cb, cr in (("R", CB_C[0], CR_C[0]), ("G", CB_C[1], CR_C[1]),
                         ("B", CB_C[2], CR_C[2])):
        m = np.zeros((128, 128), dtype=np.float32)
        m[0:64, :] = cb * PU
        m[64:128, :] = cr * PU
        putb(f"w4c{name}_b", m)

    f32c = np.zeros((128, NF32), dtype=np.float32)
    f32c[:, 0:512] = np.tile(rq.T, (16, 64))
    f32c[0::8, 512] = -np.float32(np.sqrt(8.0) * LEVEL)
    f32c[0::8, 513] = np.float32(np.sqrt(8.0) * LEVEL)

    return {"cfa": fa, "cfb": fb, "cbf": bfc.astype(bf), "cf32": f32c}


def _build_nc():
    nc = bacc.Bacc("TRN2", target_bir_lowering=False, debug=False,
                   enable_asserts=False, num_devices=N_CORES)
    x_d = nc.dram_tensor("x", [IMG_PER_CORE, 3, H, W], F32R,
                         kind="ExternalInput").ap()
    out_d = nc.dram_tensor("out", [IMG_PER_CORE, 3, H, W], BF16,
                           kind="ExternalOutput").ap()
    cfa_d = nc.dram_tensor("cfa", [128, NFA], F32R,
                           kind="ExternalInput").ap()
    cfb_d = nc.dram_tensor("cfb", [128, NFB], F32R,
                           kind="ExternalInput").ap()
    cbf_d = nc.dram_tensor("cbf", [128, NBF], BF16,
                           kind="ExternalInput").ap()
    cf32_d = nc.dram_tensor("cf32", [128, NF32], F32,
                            kind="ExternalInput").ap()

    ACT = mybir.ActivationFunctionType
    OP = mybir.AluOpType

    with tile.TileContext(nc) as tc:
        with tc.tile_pool(name="consts", bufs=1) as cp, \
             tc.tile_pool(name="xin", bufs=10) as xp, \
             tc.tile_pool(name="fwd", bufs=4) as fp, \
             tc.tile_pool(name="qnt", bufs=4) as qp, \
             tc.tile_pool(name="dcd", bufs=3) as dp, \
             tc.tile_pool(name="outp", bufs=8) as op_, \
             tc.tile_pool(name="psmm", bufs=2, space="PSUM") as pmm, \
             tc.tile_pool(name="pstp", bufs=2, space="PSUM") as ptp:

            # ---- packed const loads, all on the scalar ring in order of
            # first use: P1 weights, T1/P2 weights, q tables, bf16 weights
            cfa = cp.tile([128, NFA], F32R, tag="cfa", name="cfa")
            nc.scalar.dma_start(cfa[:], cfa_d)
            cfb = cp.tile([128, NFB], F32R, tag="cfb", name="cfb")
            nc.scalar.dma_start(cfb[:], cfb_d)
            cf32 = cp.tile([128, NF32], F32, tag="cf32", name="cf32")
            nc.scalar.dma_start(cf32[:], cf32_d)
            cbf = cp.tile([128, NBF], BF16, tag="cbf", name="cbf")
            nc.scalar.dma_start(cbf[:], cbf_d)

            def CW(name):  # [128,128] f32r const slice
                if name in OFA:
                    return cfa[:, OFA[name] * 128:OFA[name] * 128 + 128]
                return cfb[:, OFB[name] * 128:OFB[name] * 128 + 128]

            def CB(name):  # bf16 const slice
                o, n = OB[name]
                return cbf[:, o:o + n]

            rqt2 = cf32[:, 0:512]
            lneg = cf32[:, 512:513]
            lpos = cf32[:, 513:514]

            # ---- PE warm-up: release the HAM clock gate during the DMA
            # head. The PE defaults to K=4/8 (1.2 GHz) and only unthrottles
            # after ~3.4us of sustained activity; throwaway N=512 matmuls
            # over a memset tile (no DMA dependency -- PE is busy right out
            # of the framework preamble) flip it before real work arrives.
            junk = cp.tile([128, 512], BF16, tag="junk", name="junk")
            nc.gpsimd.memset(junk[:], 0.0)
            warm = ptp.tile([128, 1024], F32, tag="tp", name="pstp")
            for k in range(9):
                nc.tensor.matmul(warm[:, 0:512], junk[:, 0:128],
                                 junk[:], start=True, stop=True)

            for img in range(IMG_PER_CORE):
                # ---- batched RGB loads: one DMA per (img, t) spanning all
                # 3 channels; img 0 fans out across 3 DGE rings ----
                X = {}
                for t in range(4):
                    if img == 0 and t < 2:
                        # per-channel pieces so P1 j=0 can start on the
                        # first [128,512] to land; t0 on sync, t1 gpsimd
                        eng = nc.sync if t == 0 else nc.gpsimd
                        xt = xp.tile([128, 1536], F32R, tag="x",
                                     name=f"x_{img}_{t}")
                        for c in range(3):
                            eng.dma_start(xt[:, 512 * c:512 * (c + 1)],
                                          x_d[img, c,
                                              128 * t:128 * (t + 1), :])
                            X[c, t] = xt[:, 512 * c:512 * (c + 1)]
                        continue
                    xt = xp.tile([128, 1536], F32R, tag="x",
                                 name=f"x_{img}_{t}")
                    if img == 0:
                        eng = nc.sync if t == 2 else nc.gpsimd
                    else:
                        eng = nc.sync
                    src = x_d[img].rearrange("c (t p) w -> t p c w",
                                             p=128)[t]
                    eng.dma_start(
                        xt[:].rearrange("p (c w) -> p c w", c=3), src)
                    for c in range(3):
                        X[c, t] = xt[:, 512 * c:512 * (c + 1)]

                # ---- P1: color + H-DCT (+v-pool chroma), pairs over t ----
                d1y, d1c = [], []
                for j in range(2):
                    psY = ptp.tile([128, 1024], F32, tag="tp", name="pstp")
                    for b in range(2):
                        t = 2 * j + b
                        for c in range(3):
                            nc.tensor.matmul(psY[:, 512 * b:512 * (b + 1)],
                                             CW(f"w1y{c}"), X[c, t],
                                             start=(c == 0), stop=(c == 2))
                    ty = fp.tile([128, 1024], F32R, tag="d1y",
                                 name=f"d1y_{img}_{j}")
                    nc.scalar.activation(ty[:], psY[:], ACT.Identity,
                                         bias=lneg)
                    d1y.append(ty)
                    psC = ptp.tile([128, 1024], F32, tag="tp", name="pstp")
                    for b in range(2):
                        t = 2 * j + b
                        for c in range(3):
                            nc.tensor.matmul(psC[:, 512 * b:512 * (b + 1)],
                                             CW(f"w1c{c}"), X[c, t],
                                             start=(c == 0), stop=(c == 2))
                    tcc = fp.tile([128, 1024], F32R, tag="d1c",
                                  name=f"d1c_{img}_{j}")
                    nc.scalar.activation(tcc[:], psC[:], ACT.Copy)
                    d1c.append(tcc)

                # ---- T1: PE transposes, pairs over s ----
                t1y, t1c = [], []
                for u in range(2):
                    pty = ptp.tile([128, 1024], F32R, tag="tp", name="pstp")
                    for b in range(2):
                        s = 2 * u + b
                        for t in range(4):
                            nc.tensor.transpose(
                                pty[:, 512 * b + 128 * t:512 * b + 128 * (t + 1)],
                                d1y[t // 2][:, 512 * (t % 2) + 128 * s:
                                            512 * (t % 2) + 128 * (s + 1)],
                                CW("ident"))
                    sy = fp.tile([128, 1024], F32R, tag="t1y",
                                 name=f"t1y_{img}_{u}")
                    nc.scalar.activation(sy[:], pty[:], ACT.Copy)
                    t1y.append(sy)
                for u in range(2):
                    ptc = ptp.tile([128, 1024], F32R, tag="tp", name="pstp")
                    for b in range(2):
                        s = 2 * u + b
                        for t in range(4):
                            nc.tensor.transpose(
                                ptc[:, 512 * b + 128 * t:512 * b + 128 * (t + 1)],
                                d1c[t // 2][:, 512 * (t % 2) + 128 * s:
                                            512 * (t % 2) + 128 * (s + 1)],
                                CW("ident"))
                    sc = fp.tile([128, 1024], F32R, tag="t1c",
                                 name=f"t1c_{img}_{u}")
                    nc.scalar.activation(sc[:], ptc[:], ACT.Copy)
                    t1c.append(sc)

                # ---- P2 + quantize (all DVE) ----
                decy = []
                for u in range(2):
                    ps = pmm.tile([128, 1024], F32, tag="mm", name="psmm")
                    for b in range(2):
                        nc.tensor.matmul(ps[:, 512 * b:512 * (b + 1)],
                                         CW("w2y"),
                                         t1y[u][:, 512 * b:512 * (b + 1)],
                                         start=True, stop=True)
                    ey = qp.tile([128, 1024], F32, tag="ey",
                                 name=f"ey_{img}_{u}")
                    ry = qp.tile([128, 1024], BF16, tag="ry",
                                 name=f"ry_{img}_{u}")
                    dy = dp.tile([128, 1024], BF16, tag="decy",
                                 name=f"decy_{img}_{u}")
                    if img == IMG_PER_CORE - 1:
                        for h in range(2):
                            sl = slice(512 * h, 512 * (h + 1))
                            nc.vector.tensor_tensor(ey[:, sl], ps[:, sl],
                                                    rqt2, OP.mult)
                            nc.vector.tensor_scalar(ry[:, sl], ey[:, sl],
                                                    C_ROUND, C_ROUND,
                                                    OP.add, OP.subtract)
                            nc.vector.tensor_tensor(dy[:, sl], ry[:, sl],
                                                    CB("qt2b"), OP.mult)
                    else:
                        nc.vector.tensor_tensor(
                            ey[:].rearrange("p (b w) -> p b w", b=2),
                            ps[:].rearrange("p (b w) -> p b w", b=2),
                            rqt2.unsqueeze(1)
                            .broadcast_to([128, 2, 512]), OP.mult)
                        nc.vector.tensor_scalar(ry[:], ey[:], C_ROUND,
                                                C_ROUND, OP.add, OP.subtract)
                        nc.vector.tensor_tensor(
                            dy[:].rearrange("p (b w) -> p b w", b=2),
                            ry[:].rearrange("p (b w) -> p b w", b=2),
                            CB("qt2b").unsqueeze(1)
                            .broadcast_to([128, 2, 512]), OP.mult)
                    decy.append(dy)

                psc = pmm.tile([128, 1024], F32, tag="mm", name="psmm")
                for s in range(4):
                    nc.tensor.matmul(
                        psc[:, 512 * (s // 2):512 * (s // 2) + 512],
                        CW("w2c_hi" if s % 2 else "w2c_lo"),
                        t1c[s // 2][:, 512 * (s % 2):512 * (s % 2) + 512],
                        start=(s % 2 == 0), stop=(s % 2 == 1))
                ec = qp.tile([128, 1024], F32, tag="ey", name=f"ec_{img}")
                rc = qp.tile([128, 1024], BF16, tag="ry", name=f"rc_{img}")
                decc = dp.tile([128, 1024], BF16, tag="decc",
                               name=f"decc_{img}")
                if img == IMG_PER_CORE - 1:
                    for h in range(2):
                        sl = slice(512 * h, 512 * (h + 1))
                        nc.vector.tensor_tensor(ec[:, sl], psc[:, sl],
                                                rqt2, OP.mult)
                        nc.vector.tensor_scalar(rc[:, sl], ec[:, sl],
                                                C_ROUND, C_ROUND,
                                                OP.add, OP.subtract)
                        nc.vector.tensor_tensor(decc[:, sl], rc[:, sl],
                                                CB("qt2b"), OP.mult)
                else:
                    nc.vector.tensor_tensor(
                        ec[:].rearrange("p (b w) -> p b w", b=2),
                        psc[:].rearrange("p (b w) -> p b w", b=2),
                        rqt2.unsqueeze(1)
                        .broadcast_to([128, 2, 512]), OP.mult)
                    nc.vector.tensor_scalar(rc[:], ec[:], C_ROUND, C_ROUND,
                                            OP.add, OP.subtract)
                    nc.vector.tensor_tensor(
                        decc[:].rearrange("p (b w) -> p b w", b=2),
                        rc[:].rearrange("p (b w) -> p b w", b=2),
                        CB("qt2b").unsqueeze(1)
                        .broadcast_to([128, 2, 512]), OP.mult)

                # ---- S3: fused W-IDCT + transpose (bf16 matmuls) ----
                # last image: nothing overlaps the dec-wait gaps, so the
                # HAM clock gate re-throttles mid-endgame; junk matmuls
                # interleaved into the issue stream keep the PE warm while
                # the DVE produces dec.
                def keep_warm(n):
                    if img == IMG_PER_CORE - 1:
                        w2 = ptp.tile([128, 1024], F32, tag="tp",
                                      name="pstp")
                        for _ in range(n):
                            nc.tensor.matmul(w2[:, 0:512], junk[:, 0:128],
                                             junk[:], start=True, stop=True)
                keep_warm(4)
                t2y, t2c = [], []
                for v in range(2):
                    ps = pmm.tile([128, 1024], F32, tag="mm", name="psmm")
                    keep_warm(2)
                    for b in range(2):
                        t = 2 * v + b
                        for s in range(4):
                            nc.tensor.matmul(
                                ps[:, 512 * b + 128 * s:512 * b + 128 * (s + 1)],
                                decy[s // 2][:, 512 * (s % 2) + 128 * t:
                                             512 * (s % 2) + 128 * (t + 1)],
                                CB("bdw_b"), start=True, stop=True)
                    sy = dp.tile([128, 1024], BF16, tag="t2y",
                                 name=f"t2y_{img}_{v}")
                    nc.scalar.activation(sy[:], ps[:], ACT.Identity,
                                         bias=lpos)
                    t2y.append(sy)
                for v in range(2):
                    ps = pmm.tile([128, 1024], F32, tag="mm", name="psmm")
                    keep_warm(2)
                    for b in range(2):
                        t = 2 * v + b
                        for g in range(2):
                            nc.tensor.matmul(
                                ps[:, 512 * b + 256 * g:512 * b + 256 * (g + 1)],
                                decc[:, 512 * g + 128 * t:
                                     512 * g + 128 * (t + 1)],
                                CB("pud2"), start=True, stop=True)
                    sc = dp.tile([128, 1024], BF16, tag="t2c",
                                 name=f"t2c_{img}_{v}")
                    nc.scalar.activation(sc[:], ps[:], ACT.Copy)
                    t2c.append(sc)

                # ---- P4: H-IDCT + color + clamp + store ----
                for ci, cname in enumerate(("R", "G", "B")):
                    for v in range(2):
                        if img == IMG_PER_CORE - 1 and (2 * ci + v) % 2:
                            ps = ptp.tile([128, 1024], F32, tag="tp",
                                          name="pstp")
                        else:
                            ps = pmm.tile([128, 1024], F32, tag="mm",
                                          name="psmm")
                        for b in range(2):
                            nc.tensor.matmul(
                                ps[:, 512 * b:512 * (b + 1)], CB("w4y_b"),
                                t2y[v][:, 512 * b:512 * (b + 1)],
                                start=True, stop=False)
                            nc.tensor.matmul(
                                ps[:, 512 * b:512 * (b + 1)],
                                CB(f"w4c{cname}_b"),
                                t2c[v][:, 512 * b:512 * (b + 1)],
                                start=False, stop=True)
                        og = op_.tile([128, 1024], BF16, tag="og",
                                      name=f"og_{img}_{ci}_{v}")
                        if img == IMG_PER_CORE - 1:
                            # last image: clamp+store per half; halves
                            # alternate scalar/sync rings so the tail
                            # drains on two queues
                            for b in range(2):
                                sl = slice(512 * b, 512 * (b + 1))
                                nc.vector.tensor_scalar(og[:, sl], ps[:, sl],
                                                        0.0, 1.0,
                                                        OP.max, OP.min)
                                t = 2 * v + b
                                eng = nc.gpsimd if b == 0 else nc.sync
                                eng.dma_start(
                                    out_d[img, ci, 128 * t:128 * (t + 1), :],
                                    og[:, sl])
                        else:
                            nc.vector.tensor_scalar(og[:], ps[:], 0.0, 1.0,
                                                    OP.max, OP.min)
                            nc.gpsimd.dma_start(
                                out_d[img, ci, 256 * v:256 * (v + 1), :]
                                .rearrange("(b p) w -> p b w", b=2),
                                og[:].rearrange("p (b w) -> p b w", b=2))
    nc.compile()
    return nc


_NC_CACHE = None


def kernel(input, quantize):
    global _NC_CACHE
    input = np.asarray(input, dtype=np.float32)
    quantize = np.asarray(quantize, dtype=np.float32)
    consts = _build_consts(quantize)
    if _NC_CACHE is None:
        _NC_CACHE = _build_nc()
    nc = _NC_CACHE

    in_maps = []
    for core in range(N_CORES):
        shard = np.ascontiguousarray(
            input[core * IMG_PER_CORE:(core + 1) * IMG_PER_CORE])
        m = {"x": shard}
        m.update(consts)
        in_maps.append(m)
    trace = bool(os.environ.get("JPEG_TRACE"))
    kw = {}
    if trace:
        kw["trace"] = True
        td = os.environ.get("JPEG_TRACE_DIR")
        if td:
            os.makedirs(td, exist_ok=True)
            kw["tmpdir"] = td
    res = bass_utils.run_bass_kernel_spmd(nc, in_maps,
                                          core_ids=list(range(N_CORES)), **kw)
    global LAST_RESULT
    LAST_RESULT = res
    out = np.concatenate(
        [np.asarray(res.results[i]["out"]) for i in range(N_CORES)], axis=0)
    return out.astype(np.float32)


LAST_RESULT = None
